# revision 34
# baseline (speedup 1.0000x reference)
"""Trainium2 Bass kernel for nn_CorrBlock: softmax(fmap1 @ fmap2.T / sqrt(D), axis=-1).

Sharding: fmap1 rows split across 8 cores (1024 rows each), fmap2 replicated.
Each core computes its [1024, 8192] slab of the output independently.

Device kernel (per core):
  - Inputs are pre-transposed on the host to [128, D/128, rows] so the
    contraction dim lands on SBUF partitions with no on-device transpose.
  - PE: matmuls accumulate the D=256 contraction in 2 chunks of 128 into PSUM.
  - ACT: Exp with fused 1/sqrt(D) scale reads PSUM, writes fp16 SBUF, and
    emits per-row partial sums via accum_out in the same pass.
  - DVE: reciprocal of the row sum, then per-row scalar multiply in fp16
    (2x/4x DVE mode).
  - DMA out the normalized [128, 8192] block as fp16; the host upcasts to
    fp32. fp16 quantization of softmax probs is ~5e-4 rel err, far inside
    tolerance, and halves the dominant output DMA traffic (33.5 -> 16.8 MB
    per core).
"""

import os
import sys

import numpy as np

if "/opt/trn_rl_repo" not in sys.path:
    sys.path.insert(0, "/opt/trn_rl_repo")

import concourse.bacc as bacc
import concourse.bass as bass
import concourse.mybir as mybir
import concourse.tile as tile
from concourse.bass_utils import run_bass_kernel_spmd

N, M, D = 8192, 8192, 256
N_CORES = 8
NB = N // N_CORES  # rows per core
DC = D // 128  # contraction chunks
QC = 2048  # columns handled per PSUM tile (4 banks)

# Matmul input dtype: "float16" halves input DMA bytes and doubles PE rate
# vs "float32r", at ~5e-4 softmax rel err (vs ~2e-4). Both are far inside
# tolerance; float16 wins on the DMA roofline.
MM_DT = os.environ.get("CORR_MM_DT", "float16")

# Populated by kernel() on every run (exec_time_ns only when tracing).
last_run_info: dict = {}


def _chunks(m):
    """Uniform 2048-wide column chunks (4 PSUM banks each). Finer splits
    were tried and regressed: the extra per-ACTIVATE overhead pushed the
    scalar engine past the DMA drain pace and starved the output stream."""
    if m % 2048:
        return [m]
    return [2048] * (m // 2048)


def build_nc(nb=NB, m=M, dc=DC, qc=QC, mm_dt=None, exp_bufs=None):
    """Build the per-core Bass program. Shapes in elements."""
    f32 = mybir.dt.float32
    f16 = mybir.dt.float16
    mm_dtype = getattr(mybir.dt, mm_dt or MM_DT)
    if exp_bufs is None:
        exp_bufs = 6
    n_blocks = nb // 128
    chunks = _chunks(m)
    n_q = len(chunks)
    coff = [sum(chunks[:i]) for i in range(n_q + 1)]  # column offsets
    scale = 1.0 / (D**0.5)

    nc = bacc.Bacc("TRN2", target_bir_lowering=False, debug=False)

    f1t = nc.dram_tensor("f1t", [128, dc, nb], mm_dtype, kind="ExternalInput")
    f2t = nc.dram_tensor("f2t", [128, dc, m], mm_dtype, kind="ExternalInput")
    out = nc.dram_tensor("out", [nb, m], f16, kind="ExternalOutput")


    with tile.TileContext(nc) as tc:
        with (
            tc.tile_pool(name="weights", bufs=1) as wpool,
            tc.tile_pool(name="exps", bufs=n_blocks) as epool,
            tc.tile_pool(name="stats", bufs=n_blocks) as spool,
            tc.tile_pool(name="psum", bufs=2, space="PSUM") as ppool,
        ):
            # Stage the input DMAs so the first matmul chunk can start as
            # early as possible: block-0 slice of f1 first (tiny), then the
            # first 512 cols of f2 (feeds matmul j=0), then the rest.
            # Stage the input DMAs so the first matmul chunk can start as
            # early as possible: block-0 slice of f1 first (tiny), then the
            # first 512 cols of f2 (feeds matmul j=0), then the rest.
            f1s = wpool.tile([128, dc, nb], mm_dtype, tag="f1s")
            nc.sync.dma_start(f1s[:, :, 0:128], f1t[:, :, 0:128])
            f2s = []
            f2q0a = wpool.tile([128, dc, 512], mm_dtype, tag="f2q0a", name="f2q0a")
            nc.sync.dma_start(f2q0a[:], f2t[:, :, 0:512])
            f2q0b = wpool.tile(
                [128, dc, chunks[0] - 512], mm_dtype, tag="f2q0b", name="f2q0b"
            )
            nc.sync.dma_start(f2q0b[:], f2t[:, :, 512 : chunks[0]])
            nc.sync.dma_start(f1s[:, :, 128:nb], f1t[:, :, 128:nb])
            for q in range(1, n_q):
                f2q = wpool.tile(
                    [128, dc, chunks[q]], mm_dtype, tag=f"f2q_{q}", name=f"f2q_{q}"
                )
                nc.sync.dma_start(f2q[:], f2t[:, :, coff[q] : coff[q + 1]])
                f2s.append(f2q)

            def rhs_slice(c, d):
                """RHS AP for the 512-wide matmul column group at global col c."""
                if c < 512:
                    return f2q0a[:, d, :]
                if c < chunks[0]:
                    return f2q0b[:, d, c - 512 : c]
                q = c // qc
                r = c % qc
                return f2s[q - 1][:, d, r : r + 512]

            # Warm up the PE p-state while f2 streams in: dummy matmuls on
            # f1s (already resident) keep the PE clock ramping before the
    	    # first real chunk. They accumulate into the first real PSUM
            # tile, whose contents are discarded when the first real matmul
            # restarts accumulation with start=True.
            ps_first = ppool.tile(
                [128, chunks[0] // 512, 512], f32, tag="ps", name="ps_0_0"
            )
            for w in range(6):
                nc.tensor.matmul(
                    ps_first[:, 0, 0:128],
                    f1s[:, 0, 0:128],
                    f1s[:, 0, 0:128],
                    start=(w == 0),
                    stop=False,
                )

            # Two-phase column schedule. Phase A covers chunk q0 of every
            # block while the remaining f2 chunks stream in; phase B covers
            # q1..q3 block by block, normalizing and draining each block's
            # output as soon as its row is complete. This keeps ACT gap-free
            # from the first chunk and spreads the output DMA over ~3/4 of
            # the run instead of piling it at the end. Block 0's first chunk
            # is split 512+1536 so the very first activation starts as soon
            # as the first 512 f2 columns land.
            exps_t, sums_t, rsum_t, recip_t = [], [], [], []
            for b in range(n_blocks):
                exps_t.append(epool.tile([128, m], f16, tag="exps", name=f"exps_{b}"))
                sums_t.append(
                    spool.tile([128, n_q + 1], f32, tag="sums", name=f"sums_{b}")
                )
                rsum_t.append(spool.tile([128, 1], f32, tag="rsum", name=f"rsum_{b}"))
                recip_t.append(
                    spool.tile([128, 1], f32, tag="recip", name=f"recip_{b}")
                )
            n_sums = [0] * n_blocks  # accumulator columns used per block

            def do_cols(b, c0, c1, ps=None):
                """Matmul columns [c0,c1) of block b + one Exp activation."""
                n_j = (c1 - c0) // 512
                if ps is None:
                    ps = ppool.tile(
                        [128, n_j, 512], f32, tag="ps", name=f"ps_{b}_{c0}"
                    )
                for d in range(dc):
                    lhsT = f1s[:, d, b * 128 : (b + 1) * 128]
                    for j in range(n_j):
                        nc.tensor.matmul(
                            ps[:, j, :],
                            lhsT,
                            rhs_slice(c0 + j * 512, d),
                            start=(d == 0),
                            stop=(d == dc - 1),
                        )
                k = n_sums[b]
                n_sums[b] += 1
                nc.scalar.activation(
                    exps_t[b][:, c0:c1],
                    ps.rearrange("p a b -> p (a b)"),
                    mybir.ActivationFunctionType.Exp,
                    scale=scale,
                    accum_out=sums_t[b][:, k : k + 1],
                )

            def normalize_and_store(b):
                exps = exps_t[b]
                rsum, recip = rsum_t[b], recip_t[b]
                nc.vector.reduce_sum(
                    rsum[:], sums_t[b][:, 0 : n_sums[b]], axis=mybir.AxisListType.X
                )
                nc.vector.reciprocal(recip[:], rsum[:])
                for q in range(n_q):
                    sl = slice(coff[q], coff[q + 1])
                    nc.vector.tensor_scalar_mul(exps[:, sl], exps[:, sl], recip[:])
                    nc.sync.dma_start(out[b * 128 : (b + 1) * 128, sl], exps[:, sl])

            # Phase A: chunk q0 of each block carries the input-DMA window
            # (only the first f2 chunk is needed for 16us of ACT work).
            # Block 0's q1..q3 are interleaved in as their f2 chunks land,
            # so its normalized output starts draining the DMA queues ~6us
            # earlier — the output stream (46.6us) otherwise has zero slack
            # in its window and every jitter accumulates into the tail.
            do_cols(0, 0, chunks[0], ps=ps_first)
            do_cols(1, 0, chunks[0])
            do_cols(2, 0, chunks[0])
            do_cols(0, coff[1], coff[2])
            do_cols(3, 0, chunks[0])
            do_cols(0, coff[2], coff[3])
            do_cols(4, 0, chunks[0])
            do_cols(0, coff[3], coff[4])
            normalize_and_store(0)
            for b in range(5, n_blocks):
                do_cols(b, 0, chunks[0])

            # Phase B: chunks q1..q3 per block, then normalize + store.
            for b in range(1, n_blocks):
                for q in range(1, n_q):
                    do_cols(b, coff[q], coff[q + 1])
                normalize_and_store(b)

    nc.compile()
    return nc


_nc_cache: dict = {}


def _get_nc():
    key = MM_DT
    if key not in _nc_cache:
        _nc_cache[key] = build_nc()
    return _nc_cache[key]


def kernel(fmap1: np.ndarray, fmap2: np.ndarray) -> np.ndarray:
    f1 = np.asarray(fmap1, dtype=np.float32)
    f2 = np.asarray(fmap2, dtype=np.float32)
    np_mm = mybir.dt.np(getattr(mybir.dt, MM_DT))
    # [rows, D] -> [128, D/128, rows]: f1t[dp, dcc, n] = f1[n, dcc*128 + dp]
    f1t = np.ascontiguousarray(
        f1.T.reshape(DC, 128, N).transpose(1, 0, 2).astype(np_mm)
    )
    f2t = np.ascontiguousarray(
        f2.T.reshape(DC, 128, M).transpose(1, 0, 2).astype(np_mm)
    )

    nc = _get_nc()
    in_maps = [
        {"f1t": np.ascontiguousarray(f1t[:, :, i * NB : (i + 1) * NB]), "f2t": f2t}
        for i in range(N_CORES)
    ]
    trace = bool(os.environ.get("BASS_TRACE"))
    res = run_bass_kernel_spmd(nc, in_maps, list(range(N_CORES)), trace=trace)
    last_run_info.clear()
    last_run_info.update(
        exec_time_ns=res.exec_time_ns,
        mean_exec_time_ns=res.mean_exec_time_ns,
        profile_json=res.profile_json,
        trace_path=(res.instructions_and_trace or (None, None))[1],
    )
    return np.concatenate(
        [res.results[i]["out"] for i in range(N_CORES)], axis=0
    ).astype(np.float32)

